# revision 38
# baseline (speedup 1.0000x reference)
"""Trainium2 Bass kernel: 4096x4096 valid cross-correlation with an 11x11
filter + scalar bias, sharded row-wise across 8 NeuronCores.

Strategy
--------
Host-side sharding (halo = overlapping row slices, no collectives): core m
gets input rows [512m, 512m + 522) (core 7 shifted up to stay in bounds)
and produces output rows [512m, 512m + 512).

Per-core compute: conv expressed as banded matmuls on the TensorEngine.
For each kernel column dj, a banded stationary matrix
    B_dj[k, m] = w[k - m, dj]   (0 <= k - m < 11)
contracts over 128 image rows, while column-shifted slices of the image
slab stream as the moving operand:
    out[m, n] += sum_k B_dj[k, m] * x[r0 + k, n0 + n + dj]
Accumulating the 11 dj-shifted matmuls in one PSUM bank yields the full
11x11 correlation for a [118, 512] output tile.

vs the f32r baseline (138.3us -> ~116us measured; matmul stream ends
~98us):
- bf16 operands: ~217 ns vs ~240 ns sustained per 512-col matmul on HW
  (less clock throttle), input DMA halves. Host casts x -> bf16.
- bf16 output + host cast back to f32: store traffic halves.
- slab 0 loaded in one column chunk per bank (subtile DMA deps) so the
  first matmul group starts as soon as the NEFF startup barrier (~7us)
  and the first 0.13MB chunk allow, not after the whole 1MB slab.
- the 40-row tail runs as 2-way PE row tiling (tile_position (0,0) and
  (64,0), K=50): both tiles stream concurrently (measured 2 matmuls per
  216ns), halving the tail's streamed columns vs a 128-mode pass.
  (Column tiling, tile_position[1]=64, is broken in this walrus build:
  ISA check s3d3_mm_valid_dst_partition rejects its encoding.)
- acts per [M, 512] psum bank; wide multi-KB-row stores (half-slab and
  per-pair strips) because DMA efficiency needs >=2KB rows and each
  store's HBM-write completion ack costs ~3-15us; the last outstanding
  write at block end is one small strip.
"""

import os
import sys

import numpy as np

for _p in ("/opt/trn_rl_repo", "/root/.axon_site/_ro/trn_rl_repo"):
    if os.path.isdir(_p) and _p not in sys.path:
        sys.path.insert(0, _p)

_jp = os.environ.get("JAX_PLATFORMS", "")
if "axon" not in _jp.split(","):
    os.environ["JAX_PLATFORMS"] = ("axon," + _jp).strip(",")

import ml_dtypes
import concourse.bacc as bacc
import concourse.bass as bass
import concourse.mybir as mybir
import concourse.tile as tile
from concourse.bass_utils import run_bass_kernel_spmd

H = W = 4096
KH = KW = 11
OH = OW = H - KH + 1  # 4086
NCORES = 8
ROWS_OUT = 512            # output rows per core
ROWS_IN = ROWS_OUT + KH - 1  # 522
M_FULL = 118              # output rows per full slab (contraction K = 128)
# (x row offset, out row offset, M out rows) per full 128-row slab. The
# remaining 40-row tail (outputs 472..511, inputs 472..521) runs as 2-way
# row-tiled matmul pairs on the 64x64 PE tiling: tile (0,0) computes the
# left half-width, tile (64,0) the right half-width, concurrently — half
# the streamed columns of a full 8-bank pass.
SLABS = [(0, 0, 118), (118, 118, 118), (236, 236, 118), (354, 354, 118)]
TAIL_R0, TAIL_O0, TAIL_M = 472, 472, 40
BANK_N = [512] * 7 + [OW - 7 * 512]  # 7x512 + 502 = 4086

_cache: dict = {}
LAST_RESULT = None  # BassKernelResults of the most recent device run


def _build():
    f32 = mybir.dt.float32
    bf16 = mybir.dt.bfloat16
    nc = bacc.Bacc("TRN2", target_bir_lowering=False, debug=False,
                   num_devices=NCORES)
    xs_d = nc.dram_tensor("xs", [ROWS_IN, W], bf16, kind="ExternalInput")
    bd_d = nc.dram_tensor("bands", [128, KW * M_FULL], bf16,
                          kind="ExternalInput")
    btl_d = nc.dram_tensor("btail", [128, KW * TAIL_M], bf16,
                           kind="ExternalInput")
    bias_d = nc.dram_tensor("biasv", [1, 1], f32, kind="ExternalInput")
    out_d = nc.dram_tensor("out", [ROWS_OUT, OW], bf16, kind="ExternalOutput")

    with tile.TileContext(nc) as tc:
        with (
            tc.tile_pool(name="bp", bufs=1) as bp,
            tc.tile_pool(name="xp", bufs=1) as xp,
            tc.tile_pool(name="op", bufs=3) as op,
            tc.tile_pool(name="pp", bufs=6, space=bass.MemorySpace.PSUM) as pp,
            tc.tile_pool(name="pw", bufs=1, space=bass.MemorySpace.PSUM) as pw,
        ):
            # slab 0 in one column chunk per bank so bank b's matmuls are
            # gated only by chunk b's completion (~3us trigger->sem latency
            # apiece, ~0.7us apart); bank 0's chunk and the bands go first
            xt0 = xp.tile([128, W], bf16, tag="xt0", name="xt0")
            bt = bp.tile([128, KW * M_FULL], bf16, name="bt")
            bias_sb = bp.tile([1, 1], f32, name="bias_sb")
            nc.sync.dma_start(xt0[:, 0:522], xs_d.ap()[0:128, 0:522])
            nc.sync.dma_start(xt0[:, 522:1034], xs_d.ap()[0:128, 522:1034])
            nc.sync.dma_start(bt[:], bd_d.ap()[:, :])
            nc.sync.dma_start(bias_sb[:], bias_d.ap()[:, :])
            for b in range(2, 8):
                c0, c1 = 512 * b + 10, min(512 * (b + 1) + 10, W)
                nc.sync.dma_start(xt0[:, c0:c1], xs_d.ap()[0:128, c0:c1])

            # remaining slabs: whole-tile DMAs (one per slab, 16 engines)
            xts = {0: xt0}
            for si, (r0, _, _) in enumerate(SLABS):
                if si == 0:
                    continue
                xt = xp.tile([128, W], bf16, tag=f"xt{si}", name=f"xt{si}")
                nc.sync.dma_start(xt[:], xs_d.ap()[r0:r0 + 128, :])
                xts[si] = xt
            # tail slab: input rows 472..521 duplicated into both SBUF
            # partition halves for the two PE row tiles; bands likewise
            xtl = xp.tile([128, W], bf16, tag="xtl", name="xtl")
            nc.sync.dma_start(xtl[0:50, :], xs_d.ap()[TAIL_R0:ROWS_IN, :])
            nc.sync.dma_start(xtl[64:114, :], xs_d.ap()[TAIL_R0:ROWS_IN, :])
            btl = bp.tile([128, KW * TAIL_M], bf16, name="btl")
            nc.sync.dma_start(btl[:], btl_d.ap()[:, :])

            # warm the PE pstate while DMAs land; short-N matmuls on a
            # memset tile have no DMA dependency, so ramping starts at t=0
            ones_t = bp.tile([1, 128], f32, name="ones_t")
            nc.gpsimd.memset(ones_t[:], 1.0)
            warm_src = bp.tile([128, 128], bf16, name="warm_src")
            nc.vector.memset(warm_src[:], 1.0)
            # enough short-N warm iterations to keep the PE busy (and the
            # pstate ramped) until slab 0's first chunk lands ~7.5us in —
            # the ~7us NEFF startup barrier delays the first DMA trigger
            warm = pw.tile([118, 512], f32, name="warm")
            # ~109ns/iter at ramp pace: keeps the PE continuously busy
            # until just past slab 0 chunk 0's completion sem (~9.5us);
            # fewer iters lets the pstate decay and costs ~3us of slow
            # early conv matmuls (measured), while exec is unchanged
            NWARM = 44
            for i in range(NWARM):
                nc.tensor.matmul(warm[:, 0:128], warm_src[:, 0:118],
                                 warm_src[:, 0:128],
                                 start=(i == 0), stop=(i == NWARM - 1))
            bias_ps = pw.tile([128, 1], f32, name="bias_ps")
            nc.tensor.matmul(bias_ps[:], ones_t[:], bias_sb[:],
                             start=True, stop=True)
            bias_bc = bp.tile([128, 1], f32, name="bias_bc")
            nc.scalar.copy(bias_bc[:], bias_ps[:])

            # acts per bank, but one wide store per slab: a [M, 4086] bf16
            # store has 8KB rows (good per-engine DMA efficiency) and few
            # stores pay the ~3us HBM-write completion latency
            for si, (r0, o0, M) in enumerate(SLABS):
                xt = xts[si]
                ot = op.tile([M, OW], bf16, tag="ot", name=f"ot{si}")
                for b in range(8):
                    n0 = b * 512
                    N = BANK_N[b]
                    pt = pp.tile([M, 512], f32, tag="ps", name=f"ps{si}_{b}")
                    for dj in range(KW):
                        nc.tensor.matmul(
                            pt[:, :N],
                            bt[:, dj * M_FULL: dj * M_FULL + M],
                            xt[:, n0 + dj: n0 + dj + N],
                            start=(dj == 0),
                            stop=(dj == KW - 1),
                        )
                    nc.scalar.activation(
                        ot[:, n0:n0 + N], pt[:, :N],
                        mybir.ActivationFunctionType.Identity,
                        bias=bias_bc[0:M, :],
                    )
                    # half-slab stores keep HBM writes flowing during the
                    # block so little is left draining at the end
                    if b == 3:
                        nc.sync.dma_start(out_d.ap()[o0:o0 + M, 0:2048],
                                          ot[:, 0:2048])
                nc.sync.dma_start(out_d.ap()[o0:o0 + M, 2048:OW],
                                  ot[:, 2048:OW])

            # 40-row tail as 4 concurrent row-tiled bank pairs: tile (0,0)
            # streams bank 2b, tile (64,0) streams bank 2b+1 (K=50), so
            # each finished pair yields one contiguous [40, ~1024] strip
            # that is stored immediately — the final outstanding HBM write
            # is one small strip instead of the whole tail
            M = TAIL_M
            otl = op.tile([M, OW], bf16, tag="ot", name="otl")
            for b in range(4):
                nA, NB = 1024 * b, BANK_N[2 * b + 1]
                nB = 1024 * b + 512
                P = pp.tile([M, 512], f32, tag="ps", name=f"ptl{b}")
                Q = pp.tile([M, 512], f32, tag="ps", name=f"qtl{b}")
                for dj in range(KW):
                    st, sp = dj == 0, dj == KW - 1
                    nc.tensor.matmul(
                        P[:, :], btl[0:50, dj * M: dj * M + M],
                        xtl[0:50, nA + dj: nA + dj + 512],
                        start=st, stop=sp, tile_position=(0, 0))
                    nc.tensor.matmul(
                        Q[:, :NB], btl[64:114, dj * M: dj * M + M],
                        xtl[64:114, nB + dj: nB + dj + NB],
                        start=st, stop=sp, tile_position=(64, 0))
                nc.scalar.activation(
                    otl[:, nA:nA + 512], P[:, :],
                    mybir.ActivationFunctionType.Identity,
                    bias=bias_bc[0:M, :])
                nc.scalar.activation(
                    otl[:, nB:nB + NB], Q[:, :NB],
                    mybir.ActivationFunctionType.Identity,
                    bias=bias_bc[0:M, :])
                nc.sync.dma_start(
                    out_d.ap()[TAIL_O0:TAIL_O0 + M, nA:nB + NB],
                    otl[:, nA:nB + NB])
    nc.compile()
    return nc


def _bands_from_weight(weight: np.ndarray) -> np.ndarray:
    b = np.zeros((128, KW * M_FULL), np.float32)
    for dj in range(KW):
        col = weight[:, dj].astype(np.float32)
        for m in range(M_FULL):
            b[m:m + KH, dj * M_FULL + m] = col
    return b


def _tail_bands_from_weight(weight: np.ndarray) -> np.ndarray:
    # 64-row band for the PE row tiles, duplicated into both halves
    b = np.zeros((128, KW * TAIL_M), np.float32)
    for dj in range(KW):
        col = weight[:, dj].astype(np.float32)
        for m in range(TAIL_M):
            b[m:m + KH, dj * TAIL_M + m] = col
            b[64 + m:64 + m + KH, dj * TAIL_M + m] = col
    return b


def kernel(x: np.ndarray, weight: np.ndarray, bias: np.ndarray,
           _trace: bool = False, **_trace_kwargs) -> np.ndarray:
    global LAST_RESULT
    x = np.asarray(x, dtype=np.float32)
    weight = np.asarray(weight, dtype=np.float32)
    bias_v = np.asarray(bias, dtype=np.float32).reshape(1, 1)

    if "nc" not in _cache:
        _cache["nc"] = _build()
    nc = _cache["nc"]

    xb = x.astype(ml_dtypes.bfloat16)
    bands = _bands_from_weight(weight).astype(ml_dtypes.bfloat16)
    btail = _tail_bands_from_weight(weight).astype(ml_dtypes.bfloat16)
    starts = [min(m * ROWS_OUT, H - ROWS_IN) for m in range(NCORES)]
    in_maps = [
        {"xs": np.ascontiguousarray(xb[s:s + ROWS_IN]),
         "bands": bands,
         "btail": btail,
         "biasv": bias_v}
        for s in starts
    ]
    res = run_bass_kernel_spmd(nc, in_maps, core_ids=list(range(NCORES)),
                               trace=_trace, **_trace_kwargs)
    LAST_RESULT = res

    out = np.empty((OH, OW), dtype=np.float32)
    for m, s in enumerate(starts):
        r = np.asarray(res.results[m]["out"], dtype=np.float32)
        g0 = m * ROWS_OUT           # first global output row wanted from core m
        keep0 = g0 - s              # 0 for cores 0-6, 10 for core 7
        take = min(ROWS_OUT - keep0, OH - g0)
        out[g0:g0 + take] = r[keep0:keep0 + take]
    return out


# revision 44
# speedup vs baseline: 1.0166x; 1.0166x over previous
"""Trainium2 Bass kernel: 4096x4096 valid cross-correlation with an 11x11
filter + scalar bias, sharded row-wise across 8 NeuronCores.

Strategy
--------
Host-side sharding (halo = overlapping row slices, no collectives): core m
gets input rows [512m, 512m + 522) (core 7 shifted up to stay in bounds)
and produces output rows [512m, 512m + 512).

Per-core compute: conv expressed as banded matmuls on the TensorEngine.
For each kernel column dj, a banded stationary matrix
    B_dj[k, m] = w[k - m, dj]   (0 <= k - m < 11)
contracts over 128 image rows, while column-shifted slices of the image
slab stream as the moving operand:
    out[m, n] += sum_k B_dj[k, m] * x[r0 + k, n0 + n + dj]
Accumulating the 11 dj-shifted matmuls in one PSUM bank yields the full
11x11 correlation for a [118, 512] output tile.

vs the f32r baseline (138.3us -> ~116us measured; matmul stream ends
~98us):
- bf16 operands: ~217 ns vs ~240 ns sustained per 512-col matmul on HW
  (less clock throttle), input DMA halves. Host casts x -> bf16.
- bf16 output + host cast back to f32: store traffic halves.
- slab 0 loaded in one column chunk per bank (subtile DMA deps) so the
  first matmul group starts as soon as the NEFF startup barrier (~7us)
  and the first 0.13MB chunk allow, not after the whole 1MB slab.
- the 40-row tail runs as 2-way PE row tiling (tile_position (0,0) and
  (64,0), K=50): both tiles stream concurrently (measured 2 matmuls per
  216ns), halving the tail's streamed columns vs a 128-mode pass.
  (Column tiling, tile_position[1]=64, is broken in this walrus build:
  ISA check s3d3_mm_valid_dst_partition rejects its encoding.)
- acts per [M, 512] psum bank; wide multi-KB-row stores (half-slab and
  per-pair strips) because DMA efficiency needs >=2KB rows and each
  store's HBM-write completion ack costs ~3-15us; the last outstanding
  write at block end is one small strip.
"""

import os
import sys

import numpy as np

for _p in ("/opt/trn_rl_repo", "/root/.axon_site/_ro/trn_rl_repo"):
    if os.path.isdir(_p) and _p not in sys.path:
        sys.path.insert(0, _p)

_jp = os.environ.get("JAX_PLATFORMS", "")
if "axon" not in _jp.split(","):
    os.environ["JAX_PLATFORMS"] = ("axon," + _jp).strip(",")

import ml_dtypes
import concourse.bacc as bacc
import concourse.bass as bass
import concourse.mybir as mybir
import concourse.tile as tile
from concourse.bass_utils import run_bass_kernel_spmd

H = W = 4096
KH = KW = 11
OH = OW = H - KH + 1  # 4086
NCORES = 8
ROWS_OUT = 512            # output rows per core
ROWS_IN = ROWS_OUT + KH - 1  # 522
M_FULL = 118              # output rows per full slab (contraction K = 128)
# (x row offset, out row offset, M out rows) per full 128-row slab. The
# remaining 40-row tail (outputs 472..511, inputs 472..521) runs as 2-way
# row-tiled matmul pairs on the 64x64 PE tiling: tile (0,0) computes the
# left half-width, tile (64,0) the right half-width, concurrently — half
# the streamed columns of a full 8-bank pass.
SLABS = [(0, 0, 118), (118, 118, 118), (236, 236, 118), (354, 354, 118)]
TAIL_R0, TAIL_O0, TAIL_M = 472, 472, 40
BANK_N = [512] * 7 + [OW - 7 * 512]  # 7x512 + 502 = 4086

_cache: dict = {}
LAST_RESULT = None  # BassKernelResults of the most recent device run


def _build():
    f32 = mybir.dt.float32
    bf16 = mybir.dt.bfloat16
    nc = bacc.Bacc("TRN2", target_bir_lowering=False, debug=False,
                   num_devices=NCORES)
    xs_d = nc.dram_tensor("xs", [ROWS_IN, W], bf16, kind="ExternalInput")
    bd_d = nc.dram_tensor("bands", [128, KW * M_FULL], bf16,
                          kind="ExternalInput")
    btl_d = nc.dram_tensor("btail", [128, KW * TAIL_M], bf16,
                           kind="ExternalInput")
    bias_d = nc.dram_tensor("biasv", [1, 1], f32, kind="ExternalInput")
    out_d = nc.dram_tensor("out", [ROWS_OUT, OW], bf16, kind="ExternalOutput")

    with tile.TileContext(nc) as tc:
        with (
            tc.tile_pool(name="bp", bufs=1) as bp,
            tc.tile_pool(name="xp", bufs=1) as xp,
            tc.tile_pool(name="op", bufs=3) as op,
            tc.tile_pool(name="pp", bufs=6, space=bass.MemorySpace.PSUM) as pp,
            tc.tile_pool(name="pw", bufs=1, space=bass.MemorySpace.PSUM) as pw,
        ):
            # slab 0 in one column chunk per bank so bank b's matmuls are
            # gated only by chunk b's completion (~3us trigger->sem latency
            # apiece, ~0.7us apart); bank 0's chunk and the bands go first
            xt0 = xp.tile([128, W], bf16, tag="xt0", name="xt0")
            bt = bp.tile([128, KW * M_FULL], bf16, name="bt")
            bias_sb = bp.tile([1, 1], f32, name="bias_sb")
            nc.sync.dma_start(xt0[:, 0:522], xs_d.ap()[0:128, 0:522])
            nc.sync.dma_start(xt0[:, 522:1034], xs_d.ap()[0:128, 522:1034])
            nc.sync.dma_start(bt[:], bd_d.ap()[:, :])
            nc.sync.dma_start(bias_sb[:], bias_d.ap()[:, :])
            for b in range(2, 8):
                c0, c1 = 512 * b + 10, min(512 * (b + 1) + 10, W)
                nc.sync.dma_start(xt0[:, c0:c1], xs_d.ap()[0:128, c0:c1])

            # remaining slabs: whole-tile DMAs (one per slab, 16 engines)
            xts = {0: xt0}
            for si, (r0, _, _) in enumerate(SLABS):
                if si == 0:
                    continue
                xt = xp.tile([128, W], bf16, tag=f"xt{si}", name=f"xt{si}")
                nc.sync.dma_start(xt[:], xs_d.ap()[r0:r0 + 128, :])
                xts[si] = xt
            # tail slab: input rows 472..521 duplicated into both SBUF
            # partition halves for the two PE row tiles; bands likewise
            xtl = xp.tile([128, W], bf16, tag="xtl", name="xtl")
            nc.sync.dma_start(xtl[0:50, :], xs_d.ap()[TAIL_R0:ROWS_IN, :])
            nc.sync.dma_start(xtl[64:114, :], xs_d.ap()[TAIL_R0:ROWS_IN, :])
            btl = bp.tile([128, KW * TAIL_M], bf16, name="btl")
            nc.sync.dma_start(btl[:], btl_d.ap()[:, :])

            # warm the PE pstate while DMAs land; short-N matmuls on a
            # memset tile have no DMA dependency, so ramping starts at t=0
            ones_t = bp.tile([1, 128], f32, name="ones_t")
            nc.gpsimd.memset(ones_t[:], 1.0)
            warm_src = bp.tile([128, 128], bf16, name="warm_src")
            nc.vector.memset(warm_src[:], 1.0)
            # enough short-N warm iterations to keep the PE busy (and the
            # pstate ramped) until slab 0's first chunk lands ~7.5us in —
            # the ~7us NEFF startup barrier delays the first DMA trigger
            warm = pw.tile([118, 512], f32, name="warm")
            NWARM = 44
            for i in range(NWARM):
                nc.tensor.matmul(warm[:, 0:128], warm_src[:, 0:118],
                                 warm_src[:, 0:128],
                                 start=(i == 0), stop=(i == NWARM - 1))
            bias_ps = pw.tile([128, 1], f32, name="bias_ps")
            nc.tensor.matmul(bias_ps[:], ones_t[:], bias_sb[:],
                             start=True, stop=True)
            bias_bc = bp.tile([128, 1], f32, name="bias_bc")
            nc.scalar.copy(bias_bc[:], bias_ps[:])

            # acts per bank, but one wide store per slab: a [M, 4086] bf16
            # store has 8KB rows (good per-engine DMA efficiency) and few
            # stores pay the ~3us HBM-write completion latency
            for si, (r0, o0, M) in enumerate(SLABS):
                xt = xts[si]
                ot = op.tile([M, OW], bf16, tag="ot", name=f"ot{si}")
                for b in range(8):
                    n0 = b * 512
                    N = BANK_N[b]
                    pt = pp.tile([M, 512], f32, tag="ps", name=f"ps{si}_{b}")
                    for dj in range(KW):
                        nc.tensor.matmul(
                            pt[:, :N],
                            bt[:, dj * M_FULL: dj * M_FULL + M],
                            xt[:, n0 + dj: n0 + dj + N],
                            start=(dj == 0),
                            stop=(dj == KW - 1),
                        )
                    nc.scalar.activation(
                        ot[:, n0:n0 + N], pt[:, :N],
                        mybir.ActivationFunctionType.Identity,
                        bias=bias_bc[0:M, :],
                    )
                    # half-slab stores keep HBM writes flowing during the
                    # block so little is left draining at the end
                    if b == 3:
                        nc.sync.dma_start(out_d.ap()[o0:o0 + M, 0:2048],
                                          ot[:, 0:2048])
                nc.sync.dma_start(out_d.ap()[o0:o0 + M, 2048:OW],
                                  ot[:, 2048:OW])

            # 40-row tail as 4 concurrent row-tiled bank pairs: tile (0,0)
            # streams bank 2b, tile (64,0) streams bank 2b+1 (K=50), so
            # each finished pair yields one contiguous [40, ~1024] strip
            # that is stored immediately — the final outstanding HBM write
            # is one small strip instead of the whole tail
            M = TAIL_M
            otl = op.tile([M, OW], bf16, tag="ot", name="otl")
            for b in range(4):
                nA, NB = 1024 * b, BANK_N[2 * b + 1]
                nB = 1024 * b + 512
                P = pp.tile([M, 512], f32, tag="ps", name=f"ptl{b}")
                Q = pp.tile([M, 512], f32, tag="ps", name=f"qtl{b}")
                for dj in range(KW):
                    st, sp = dj == 0, dj == KW - 1
                    nc.tensor.matmul(
                        P[:, :], btl[0:50, dj * M: dj * M + M],
                        xtl[0:50, nA + dj: nA + dj + 512],
                        start=st, stop=sp, tile_position=(0, 0))
                    nc.tensor.matmul(
                        Q[:, :NB], btl[64:114, dj * M: dj * M + M],
                        xtl[64:114, nB + dj: nB + dj + NB],
                        start=st, stop=sp, tile_position=(64, 0))
                nc.scalar.activation(
                    otl[:, nA:nA + 512], P[:, :],
                    mybir.ActivationFunctionType.Identity,
                    bias=bias_bc[0:M, :])
                nc.scalar.activation(
                    otl[:, nB:nB + NB], Q[:, :NB],
                    mybir.ActivationFunctionType.Identity,
                    bias=bias_bc[0:M, :])
                nc.sync.dma_start(
                    out_d.ap()[TAIL_O0:TAIL_O0 + M, nA:nB + NB],
                    otl[:, nA:nB + NB])
    nc.compile()
    return nc


def _bands_from_weight(weight: np.ndarray) -> np.ndarray:
    b = np.zeros((128, KW * M_FULL), np.float32)
    for dj in range(KW):
        col = weight[:, dj].astype(np.float32)
        for m in range(M_FULL):
            b[m:m + KH, dj * M_FULL + m] = col
    return b


def _tail_bands_from_weight(weight: np.ndarray) -> np.ndarray:
    # 64-row band for the PE row tiles, duplicated into both halves
    b = np.zeros((128, KW * TAIL_M), np.float32)
    for dj in range(KW):
        col = weight[:, dj].astype(np.float32)
        for m in range(TAIL_M):
            b[m:m + KH, dj * TAIL_M + m] = col
            b[64 + m:64 + m + KH, dj * TAIL_M + m] = col
    return b


def kernel(x: np.ndarray, weight: np.ndarray, bias: np.ndarray,
           _trace: bool = False, **_trace_kwargs) -> np.ndarray:
    global LAST_RESULT
    x = np.asarray(x, dtype=np.float32)
    weight = np.asarray(weight, dtype=np.float32)
    bias_v = np.asarray(bias, dtype=np.float32).reshape(1, 1)

    if "nc" not in _cache:
        _cache["nc"] = _build()
    nc = _cache["nc"]

    xb = x.astype(ml_dtypes.bfloat16)
    bands = _bands_from_weight(weight).astype(ml_dtypes.bfloat16)
    btail = _tail_bands_from_weight(weight).astype(ml_dtypes.bfloat16)
    starts = [min(m * ROWS_OUT, H - ROWS_IN) for m in range(NCORES)]
    in_maps = [
        {"xs": np.ascontiguousarray(xb[s:s + ROWS_IN]),
         "bands": bands,
         "btail": btail,
         "biasv": bias_v}
        for s in starts
    ]
    res = run_bass_kernel_spmd(nc, in_maps, core_ids=list(range(NCORES)),
                               trace=_trace, **_trace_kwargs)
    LAST_RESULT = res

    out = np.empty((OH, OW), dtype=np.float32)
    for m, s in enumerate(starts):
        r = np.asarray(res.results[m]["out"], dtype=np.float32)
        g0 = m * ROWS_OUT           # first global output row wanted from core m
        keep0 = g0 - s              # 0 for cores 0-6, 10 for core 7
        take = min(ROWS_OUT - keep0, OH - g0)
        out[g0:g0 + take] = r[keep0:keep0 + take]
    return out
